# revision 1
# baseline (speedup 1.0000x reference)
"""Mamba-1 selective SSM block on 8 trn2 NeuronCores.

Sharding: 2 batch-groups x 4 channel-shards. Core c handles batch c//4 and
d_inner channels [(c%4)*512, (c%4+1)*512). Cross-core comm: AllReduce of the
x_proj partial [96, 1024] within each 4-core batch group. Host sums the 4
partial out_proj outputs per batch.

Per-core layout: big [L, d_loc, N] tensors live as 64 "group" tiles of
[128 partitions, L free] where partition p = n*8 + d_sub (n-major) covers
8 channels x 16 states. delta is replicated onto that layout by PE selector
matmuls; delta*u by transposing DMAs; the sum over states goes back through
PE selector matmuls accumulating in PSUM.
"""

import numpy as np
import ml_dtypes

import concourse.bacc as bacc
import concourse.mybir as mybir
import concourse.tile as tile
import concourse.bass as bass
from concourse import bass_utils

BF16 = mybir.dt.bfloat16
F32 = mybir.dt.float32
AF = mybir.ActivationFunctionType
OP = mybir.AluOpType

L = 1024          # sequence length
DM = 1024         # model dim
DL = 512          # local d_inner channels per core
NQ = 4            # channel chunks of 128 per core
NGRP = 64         # DL/8 groups per core
RANK = 64         # dt_rank
LH = 512          # L half for 1-bank PSUM tiles

_CACHE = {}


def _build(sim=False, reps=1):
    # sim=True: collective becomes a local copy (for TimelineSim / ablation)
    nc = bacc.Bacc("TRN2", target_bir_lowering=False, debug=False, num_devices=8)

    # inputs (per-core, host-prepped)
    xT = nc.dram_tensor("xT", [DM, L], BF16, kind="ExternalInput")
    w_in = nc.dram_tensor("w_in", [DM, 2 * DL], BF16, kind="ExternalInput")
    w_xp = nc.dram_tensor("w_xp", [DL, 96], BF16, kind="ExternalInput")
    w_dt = nc.dram_tensor("w_dt", [RANK, DL], BF16, kind="ExternalInput")
    dt_b = nc.dram_tensor("dt_b", [128, NQ], F32, kind="ExternalInput")
    w_out = nc.dram_tensor("w_out", [DL, DM], BF16, kind="ExternalInput")
    a_cols = nc.dram_tensor("a_cols", [128, NGRP], F32, kind="ExternalInput")
    d_col = nc.dram_tensor("d_col", [128, NQ], F32, kind="ExternalInput")
    convd = nc.dram_tensor("convd", [128, NQ * 4 * 128], BF16, kind="ExternalInput")
    convb = nc.dram_tensor("convb", [128, NQ], F32, kind="ExternalInput")
    selr = nc.dram_tensor("selr", [128, 16 * 128], BF16, kind="ExternalInput")
    selo = nc.dram_tensor("selo", [128, 16 * 128], BF16, kind="ExternalInput")
    selbc = nc.dram_tensor("selbc", [32, 2 * 128], BF16, kind="ExternalInput")
    out = nc.dram_tensor("out", [DM, L], F32, kind="ExternalOutput")

    with tile.TileContext(nc) as tc:
        with (
            tc.tile_pool(name="const", bufs=1) as cp,
            tc.tile_pool(name="acts", bufs=1) as ap,
            tc.tile_pool(name="wpool", bufs=1) as wp,
            tc.tile_pool(name="dram", bufs=1, space="DRAM") as dp,
            tc.tile_pool(name="grp", bufs=3) as gp,
            tc.tile_pool(name="durep", bufs=2) as drp,
            tc.tile_pool(name="ps_small", bufs=2, space="PSUM") as pss,
            tc.tile_pool(name="ps_rep", bufs=4, space="PSUM") as psr,
            tc.tile_pool(name="ps_y", bufs=2, space="PSUM") as psy,
        ):
            # ---- persistent constants/weights in SBUF ----
            xT_sb = wp.tile([128, 8 * L], BF16, tag="xT")          # [DM, L] as 8 k-chunks
            nc.sync.dma_start(xT_sb[:].rearrange("p (k l) -> p k l", k=8),
                              xT.ap().rearrange("(k p) l -> p k l", p=128))
            w_in_sb = wp.tile([128, 8 * 1024], BF16, tag="w_in")
            nc.sync.dma_start(w_in_sb[:].rearrange("p (k m) -> p k m", k=8),
                              w_in.ap().rearrange("(k p) m -> p k m", p=128))
            w_xp_sb = wp.tile([128, 4 * 96], BF16, tag="w_xp")
            nc.sync.dma_start(w_xp_sb[:].rearrange("p (k m) -> p k m", k=4),
                              w_xp.ap().rearrange("(k p) m -> p k m", p=128))
            w_dt_sb = wp.tile([64, DL], BF16, tag="w_dt")
            nc.sync.dma_start(w_dt_sb[:], w_dt.ap())
            w_out_sb = wp.tile([128, 4 * 1024], BF16, tag="w_out")
            nc.sync.dma_start(w_out_sb[:].rearrange("p (k m) -> p k m", k=4),
                              w_out.ap().rearrange("(k p) m -> p k m", p=128))
            selr_sb = cp.tile([128, 16 * 128], BF16, tag="selr")
            nc.sync.dma_start(selr_sb[:], selr.ap())
            selo_sb = cp.tile([128, 16 * 128], BF16, tag="selo")
            nc.sync.dma_start(selo_sb[:], selo.ap())
            selbc_sb = cp.tile([32, 2 * 128], BF16, tag="selbc")
            nc.sync.dma_start(selbc_sb[:], selbc.ap())
            dtb_sb = cp.tile([128, NQ], F32, tag="dtb")
            nc.sync.dma_start(dtb_sb[:], dt_b.ap())
            acol_sb = cp.tile([128, NGRP], F32, tag="acol")
            nc.sync.dma_start(acol_sb[:], a_cols.ap())
            dcol_sb = cp.tile([128, NQ], F32, tag="dcol")
            nc.sync.dma_start(dcol_sb[:], d_col.ap())
            convd_sb = cp.tile([128, NQ * 4 * 128], BF16, tag="convd")
            nc.sync.dma_start(convd_sb[:], convd.ap())
            convb_sb = cp.tile([128, NQ], F32, tag="convb")
            nc.sync.dma_start(convb_sb[:], convb.ap())

            def _mamba_body():
                # ---- activations kept across phases ----
                xin = [ap.tile([128, 3 + L], BF16, tag=f"xin{q}", name=f"xin{q}") for q in range(NQ)]
                silu_z = [ap.tile([128, L], BF16, tag=f"sz{q}", name=f"sz{q}") for q in range(NQ)]
                u = [ap.tile([128, L], BF16, tag=f"u{q}", name=f"u{q}") for q in range(NQ)]
                delta = [ap.tile([128, L], BF16, tag=f"delta{q}", name=f"delta{q}") for q in range(NQ)]
                du = [ap.tile([128, L], BF16, tag=f"du{q}", name=f"du{q}") for q in range(NQ)]
                ygate = [ap.tile([128, L], BF16, tag=f"yg{q}", name=f"yg{q}") for q in range(NQ)]
                xdbl = ap.tile([96, L], F32, tag="xdbl")
                dt_bf = ap.tile([64, L], BF16, tag="dtbf")
                bc_bf = ap.tile([32, L], BF16, tag="bcbf")
                b_rep = ap.tile([128, L], BF16, tag="brep")
                c_rep = ap.tile([128, L], BF16, tag="crep")

                for q in range(NQ):
                    nc.vector.memset(xin[q][:, 0:3], 0.0)

                # ---- phase 1: in_proj (xz = W_in^T-slice @ x) ----
                # out[m_tile, l] accumulated over 8 k-chunks; m 0..3 -> xin, 4..7 -> z
                for m in range(8):
                    for lh in range(2):
                        ps = pss.tile([128, LH], F32, tag="ps")
                        for k in range(8):
                            nc.tensor.matmul(
                                ps[:],
                                w_in_sb[:, m * 128 + k * 1024:(m + 1) * 128 + k * 1024],
                                xT_sb[:, k * L + lh * LH:k * L + lh * LH + LH],
                                start=(k == 0), stop=(k == 7),
                            )
                        if m < 4:
                            nc.scalar.copy(xin[m][:, 3 + lh * LH:3 + lh * LH + LH], ps[:])
                        else:
                            nc.scalar.activation(
                                silu_z[m - 4][:, lh * LH:lh * LH + LH], ps[:], AF.Silu)

                # ---- phase 2: causal conv (width 4) on PE + silu -> u ----
                # xc = sum_k diag(w_k) @ xin[:, k:k+L]; u = silu(xc + bias)
                for q in range(NQ):
                    for lh in range(2):
                        ps = pss.tile([128, LH], F32, tag="ps")
                        for k in range(4):
                            nc.tensor.matmul(
                                ps[:],
                                convd_sb[:, (q * 4 + k) * 128:(q * 4 + k + 1) * 128],
                                xin[q][:, k + lh * LH:k + lh * LH + LH],
                                start=(k == 0), stop=(k == 3))
                        nc.scalar.activation(
                            u[q][:, lh * LH:lh * LH + LH], ps[:], AF.Silu,
                            bias=convb_sb[:, q:q + 1])

                # ---- phase 3: x_proj partial + AllReduce ----
                for lh in range(2):
                    ps = pss.tile([96, LH], F32, tag="ps")
                    for q in range(NQ):
                        nc.tensor.matmul(
                            ps[:], w_xp_sb[:, q * 96:(q + 1) * 96],
                            u[q][:, lh * LH:lh * LH + LH],
                            start=(q == 0), stop=(q == 3))
                    nc.scalar.copy(xdbl[:, lh * LH:lh * LH + LH], ps[:])
                du_d = [dp.tile([128, L], BF16, tag=f"du_d{q}", name=f"du_d{q}")
                        for q in range(NQ)]
                cc_in = dp.tile([96, L], F32, name="cc_in")
                cc_out = dp.tile([96, L], F32, name="cc_out")
                nc.sync.dma_start(cc_in[:], xdbl[:])
                if sim:
                    nc.sync.dma_start(cc_out[:], cc_in[:])
                else:
                    nc.gpsimd.collective_compute(
                        "AllReduce", OP.add,
                        replica_groups=[[0, 1, 2, 3], [4, 5, 6, 7]],
                        ins=[cc_in.opt()], outs=[cc_out.opt()])
                nc.sync.dma_start(xdbl[:], cc_out[:])

                # split: dt rows 0:64, B 64:80, C 80:96; replicate B/C onto the
                # n-major (n*8+d) layout via a small selector matmul
                nc.vector.tensor_copy(dt_bf[:], xdbl[0:64, :])
                nc.vector.tensor_copy(bc_bf[:], xdbl[64:96, :])
                for which, dest in ((0, b_rep), (1, c_rep)):
                    for lh in range(2):
                        ps = pss.tile([128, LH], F32, tag="ps")
                        nc.tensor.matmul(
                            ps[:], selbc_sb[:, which * 128:(which + 1) * 128],
                            bc_bf[:, lh * LH:lh * LH + LH], start=True, stop=True)
                        nc.scalar.copy(dest[:, lh * LH:lh * LH + LH], ps[:])

                # ---- phase 4: delta = softplus(dt_proj @ dt + b); du = delta*u ----
                for q in range(NQ):
                    for lh in range(2):
                        ps = pss.tile([128, LH], F32, tag="ps")
                        nc.tensor.matmul(
                            ps[:], w_dt_sb[:, q * 128:(q + 1) * 128],
                            dt_bf[:, lh * LH:lh * LH + LH], start=True, stop=True)
                        # softplus(x+b) = ln(1 + exp(x+b)); Softplus has no act table
                        spe = gp.tile([128, LH], F32, tag="spe")
                        nc.scalar.activation(
                            spe[:], ps[:], AF.Exp, bias=dtb_sb[:, q:q + 1])
                        nc.scalar.activation(
                            delta[q][:, lh * LH:lh * LH + LH], spe[:], AF.Ln, bias=1.0)
                    nc.vector.tensor_tensor(
                        du[q][:], delta[q][:], u[q][:], op=OP.mult)
                    nc.sync.dma_start(du_d[q][:], du[q][:])

                # ---- phase 5: per-group SSM scan ----
                for q in range(NQ):
                    yps = [psy.tile([128, LH], F32, tag="ps_y", name=f"yps{q}_{i}")
                           for i in range(2)]
                    dureps = []
                    for hf in range(2):
                        # replicate du half-chunk onto (n,d) partitions for 8
                        # groups: durep[n*8+d, j*L+l] = du[q][(hf*8+j)*8+d, l]
                        durep = drp.tile([128, 8 * L], BF16, tag="durep",
                                         name=f"durep{q}_{hf}")
                        srcv = du_d[q][hf * 64:(hf + 1) * 64, :].rearrange(
                            "(j d) l -> d j l", d=8)
                        for n in range(16):
                            nc.sync.dma_start(
                                durep[n * 8:(n + 1) * 8, :].rearrange(
                                    "d (j l) -> d j l", j=8),
                                srcv)
                        # bu = durep * B (B broadcast over group-blocks), in place
                        nc.vector.tensor_tensor(
                            durep[:].rearrange("p (j l) -> p j l", j=8),
                            durep[:].rearrange("p (j l) -> p j l", j=8),
                            b_rep[:].unsqueeze(1).broadcast_to([128, 8, L]),
                            op=OP.mult)
                        dureps.append(durep)

                    for j in range(16):
                        g = q * 16 + j
                        durep = dureps[j // 8]
                        a_t = gp.tile([128, L], BF16, tag="a", name=f"a{g}")
                        h_t = gp.tile([128, L], BF16, tag="h", name=f"h{g}")
                        g_t = gp.tile([128, L], BF16, tag="g", name=f"g{g}")
                        for lh in range(2):
                            psd = psr.tile([128, LH], F32, tag="ps_rep")
                            nc.tensor.matmul(
                                psd[:], selr_sb[:, j * 128:(j + 1) * 128],
                                delta[q][:, lh * LH:lh * LH + LH], start=True, stop=True)
                            nc.scalar.activation(
                                a_t[:, lh * LH:lh * LH + LH], psd[:], AF.Exp,
                                bias=0.0, scale=acol_sb[:, g:g + 1])
                        nc.vector.tensor_tensor_scan(
                            h_t[:], a_t[:], durep[:, (j % 8) * L:(j % 8 + 1) * L],
                            0.0, OP.mult, OP.add)
                        nc.vector.tensor_tensor(g_t[:], h_t[:], c_rep[:], op=OP.mult)
                        # reduce over states into y accumulator for chunk q
                        for lh in range(2):
                            nc.tensor.matmul(
                                yps[lh][:], selo_sb[:, j * 128:(j + 1) * 128],
                                g_t[:, lh * LH:lh * LH + LH],
                                start=(j == 0), stop=(j == 15), skip_group_check=True)
                    for lh in range(2):
                        t1 = gp.tile([128, LH], F32, tag="t1")
                        nc.vector.scalar_tensor_tensor(
                            t1[:], u[q][:, lh * LH:lh * LH + LH],
                            dcol_sb[:, q:q + 1], yps[lh][:],
                            op0=OP.mult, op1=OP.add)
                        nc.vector.tensor_tensor(
                            ygate[q][:, lh * LH:lh * LH + LH], t1[:],
                            silu_z[q][:, lh * LH:lh * LH + LH], op=OP.mult)

                # ---- phase 6: out_proj ----
                for m in range(8):
                    for lh in range(2):
                        ps = pss.tile([128, LH], F32, tag="ps")
                        for q in range(NQ):
                            nc.tensor.matmul(
                                ps[:], w_out_sb[:, q * 1024 + m * 128:q * 1024 + (m + 1) * 128],
                                ygate[q][:, lh * LH:lh * LH + LH],
                                start=(q == 0), stop=(q == 3))
                        ot = gp.tile([128, LH], F32, tag="ot")
                        nc.scalar.copy(ot[:], ps[:])
                        nc.sync.dma_start(
                            out.ap()[m * 128:(m + 1) * 128, lh * LH:lh * LH + LH], ot[:])

            for _rep in range(reps):
                _mamba_body()

    nc.compile()
    return nc


def _prep_core_inputs(c, x, in_proj_w, conv_w, conv_b, x_proj_w, dt_proj_w,
                      dt_proj_b, A_log, D, out_proj_w, sel_r, sel_o, sel_bc):
    b, s = divmod(c, 4)
    sl = slice(s * DL, (s + 1) * DL)
    bf = ml_dtypes.bfloat16
    A = (-np.exp(A_log[sl])).astype(np.float32)            # [512, 16]
    # a_cols[:, g][p] = A[g*8 + p%8, p//8]  (n-major partitions)
    a_cols = np.empty((128, NGRP), np.float32)
    p = np.arange(128)
    for g in range(NGRP):
        a_cols[:, g] = A[g * 8 + (p % 8), p // 8]
    w_in_loc = np.concatenate([in_proj_w[sl], in_proj_w[2048 + s * DL:2048 + (s + 1) * DL]], 0)
    # conv as 16 diagonal [128,128] matrices (chunk-major, tap-minor)
    convd = np.zeros((128, NQ * 4 * 128), np.float32)
    cw = conv_w[sl, 0, :]                                  # [512, 4]
    for q in range(NQ):
        for k in range(4):
            blk = (q * 4 + k) * 128
            convd[np.arange(128), blk + np.arange(128)] = cw[q * 128:(q + 1) * 128, k]
    return {
        "xT": np.ascontiguousarray(x[b].T).astype(bf),
        "w_in": np.ascontiguousarray(w_in_loc.T).astype(bf),
        "w_xp": np.ascontiguousarray(x_proj_w[:, sl].T).astype(bf),
        "w_dt": np.ascontiguousarray(dt_proj_w[sl].T).astype(bf),
        "dt_b": np.ascontiguousarray(dt_proj_b[sl].reshape(NQ, 128).T).astype(np.float32),
        "w_out": np.ascontiguousarray(out_proj_w[:, sl].T).astype(bf),
        "a_cols": a_cols,
        "d_col": np.ascontiguousarray(D[sl].reshape(NQ, 128).T).astype(np.float32),
        "convd": convd.astype(bf),
        "convb": np.ascontiguousarray(conv_b[sl].reshape(NQ, 128).T).astype(np.float32),
        "selr": sel_r,
        "selo": sel_o,
        "selbc": sel_bc,
    }


def _selectors():
    bf = ml_dtypes.bfloat16
    p = np.arange(128)
    sel_r = np.zeros((128, 16 * 128), dtype=bf)
    sel_o = np.zeros((128, 16 * 128), dtype=bf)
    for j in range(16):
        sel_r[j * 8 + (p % 8), j * 128 + p] = 1       # replicate 8 ch -> (n,d)
        sel_o[p, j * 128 + j * 8 + (p % 8)] = 1       # reduce states back
    sel_bc = np.zeros((32, 2 * 128), dtype=bf)
    sel_bc[p // 8, p] = 1                              # B: rows 0:16 -> n-major
    sel_bc[16 + p // 8, 128 + p] = 1                   # C: rows 16:32
    return sel_r, sel_o, sel_bc


def kernel(x, in_proj_w, conv_w, conv_b, x_proj_w, dt_proj_w, dt_proj_b,
           A_log, D, out_proj_w):
    sel_r, sel_o, sel_bc = _selectors()
    if "nc" not in _CACHE:
        _CACHE["nc"] = _build()
    nc = _CACHE["nc"]

    args = (x, in_proj_w, conv_w, conv_b, x_proj_w, dt_proj_w, dt_proj_b,
            A_log, D, out_proj_w)
    in_maps = [_prep_core_inputs(c, *args, sel_r, sel_o, sel_bc) for c in range(8)]
    res = bass_utils.run_bass_kernel_spmd(nc, in_maps, core_ids=list(range(8)))
    outs = res.results
    _CACHE["last_result"] = res

    full = np.zeros((2, L, DM), dtype=np.float32)
    for b in range(2):
        acc = outs[4 * b]["out"].astype(np.float32).copy()
        for s in range(1, 4):
            acc += outs[4 * b + s]["out"]
        full[b] = acc.T
    return full



# revision 6
# speedup vs baseline: 1.1258x; 1.1258x over previous
"""Mamba-1 selective SSM block on 8 trn2 NeuronCores.

Sharding: 2 batch-groups x 4 channel-shards. Core c handles batch c//4 and
d_inner channels [(c%4)*512, (c%4+1)*512). Cross-core comm: AllReduce of the
x_proj partial [96, 512] per L-half within each 4-core batch group. Host sums
the 4 partial out_proj outputs per batch.

v2 structure: the whole network is software-pipelined over two L-halves so
the AllReduce latency hides under compute of the other half. The big
[L, d_loc, N] state tensors live as group tiles of [128 partitions, 8*LH]
where partition p = n*8 + d_sub covers 8 channels x 16 states and the free
dim concatenates 8 groups' L-half segments; one DVE scan per (q, hf, half)
covers 8 groups using a=0 segment resets (carry across halves is folded into
the first bu element of each segment). delta and du replication onto (n,d)
both go through PE selector matmuls (same stationary) with Act eviction.
"""

import numpy as np
import ml_dtypes

import concourse.bacc as bacc
import concourse.mybir as mybir
import concourse.tile as tile
import concourse.bass as bass
from concourse import bass_utils

BF16 = mybir.dt.bfloat16
F32 = mybir.dt.float32
AF = mybir.ActivationFunctionType
OP = mybir.AluOpType

L = 1024          # sequence length
DM = 1024         # model dim
DL = 512          # local d_inner channels per core
NQ = 4            # channel chunks of 128 per core
NGRP = 64         # DL/8 groups per core
RANK = 64         # dt_rank
LH = 512          # L half

_CACHE = {}


def _build(sim=False):
    nc = bacc.Bacc("TRN2", target_bir_lowering=False, debug=False, num_devices=8)

    # inputs (per-core, host-prepped)
    xT = nc.dram_tensor("xT", [DM, L], BF16, kind="ExternalInput")
    w_in = nc.dram_tensor("w_in", [DM, 2 * DL], BF16, kind="ExternalInput")
    w_xp = nc.dram_tensor("w_xp", [DL, 96], BF16, kind="ExternalInput")
    w_dt = nc.dram_tensor("w_dt", [RANK, DL], BF16, kind="ExternalInput")
    dt_b = nc.dram_tensor("dt_b", [128, NQ], F32, kind="ExternalInput")
    w_out = nc.dram_tensor("w_out", [DL, DM], BF16, kind="ExternalInput")
    a_cols = nc.dram_tensor("a_cols", [128, NGRP], F32, kind="ExternalInput")
    d_col = nc.dram_tensor("d_col", [128, NQ], F32, kind="ExternalInput")
    convd = nc.dram_tensor("convd", [128, NQ * 4 * 128], BF16, kind="ExternalInput")
    convb = nc.dram_tensor("convb", [128, NQ], F32, kind="ExternalInput")
    selr = nc.dram_tensor("selr", [128, 16 * 128], BF16, kind="ExternalInput")
    selo = nc.dram_tensor("selo", [128, 16 * 128], BF16, kind="ExternalInput")
    selbc = nc.dram_tensor("selbc", [32, 2 * 128], BF16, kind="ExternalInput")
    out = nc.dram_tensor("out", [DM, L], BF16, kind="ExternalOutput")

    with tile.TileContext(nc) as tc:
        with (
            tc.tile_pool(name="const", bufs=1) as cp,
            tc.tile_pool(name="acts", bufs=1) as ap,
            tc.tile_pool(name="wpool", bufs=1) as wp,
            tc.tile_pool(name="dram", bufs=1, space="DRAM") as dp,
            tc.tile_pool(name="grp", bufs=2) as gp,
            tc.tile_pool(name="sa", bufs=2) as sa,
            tc.tile_pool(name="ps_small", bufs=2, space="PSUM") as pss,
            tc.tile_pool(name="ps_rep", bufs=4, space="PSUM") as psr,
            tc.tile_pool(name="ps_y", bufs=2, space="PSUM") as psy,
        ):
            # ---- persistent constants/weights in SBUF (critical-path first) ----
            xT_sb = wp.tile([128, 8 * L], BF16, tag="xT")          # [DM, L] as 8 k-chunks
            nc.sync.dma_start(xT_sb[:].rearrange("p (k l) -> p k l", k=8),
                              xT.ap().rearrange("(k p) l -> p k l", p=128))
            w_in_sb = wp.tile([128, 8 * 1024], BF16, tag="w_in")
            nc.sync.dma_start(w_in_sb[:].rearrange("p (k m) -> p k m", k=8),
                              w_in.ap().rearrange("(k p) m -> p k m", p=128))
            convd_sb = cp.tile([128, NQ * 4 * 128], BF16, tag="convd")
            nc.sync.dma_start(convd_sb[:], convd.ap())
            convb_sb = cp.tile([128, NQ], F32, tag="convb")
            nc.sync.dma_start(convb_sb[:], convb.ap())
            w_xp_sb = wp.tile([128, 4 * 96], BF16, tag="w_xp")
            nc.sync.dma_start(w_xp_sb[:].rearrange("p (k m) -> p k m", k=4),
                              w_xp.ap().rearrange("(k p) m -> p k m", p=128))
            w_dt_sb = wp.tile([64, DL], BF16, tag="w_dt")
            nc.sync.dma_start(w_dt_sb[:], w_dt.ap())
            selr_sb = cp.tile([128, 16 * 128], BF16, tag="selr")
            nc.sync.dma_start(selr_sb[:], selr.ap())
            selo_sb = cp.tile([128, 16 * 128], BF16, tag="selo")
            nc.sync.dma_start(selo_sb[:], selo.ap())
            selbc_sb = cp.tile([32, 2 * 128], BF16, tag="selbc")
            nc.sync.dma_start(selbc_sb[:], selbc.ap())
            dtb_sb = cp.tile([128, NQ], F32, tag="dtb")
            nc.sync.dma_start(dtb_sb[:], dt_b.ap())
            acol_sb = cp.tile([128, NGRP], F32, tag="acol")
            nc.sync.dma_start(acol_sb[:], a_cols.ap())
            dcol_sb = cp.tile([128, NQ], F32, tag="dcol")
            nc.sync.dma_start(dcol_sb[:], d_col.ap())
            w_out_sb = wp.tile([128, 4 * 1024], BF16, tag="w_out")
            nc.sync.dma_start(w_out_sb[:].rearrange("p (k m) -> p k m", k=4),
                              w_out.ap().rearrange("(k p) m -> p k m", p=128))

            # ---- activations persistent across halves ----
            xin = [ap.tile([128, 3 + L], BF16, tag=f"xin{q}", name=f"xin{q}") for q in range(NQ)]
            silu_z = [ap.tile([128, L], BF16, tag=f"sz{q}", name=f"sz{q}") for q in range(NQ)]
            u = [ap.tile([128, L], BF16, tag=f"u{q}", name=f"u{q}") for q in range(NQ)]
            delta = [ap.tile([128, L], BF16, tag=f"delta{q}", name=f"delta{q}") for q in range(NQ)]
            du = [ap.tile([128, L], BF16, tag=f"du{q}", name=f"du{q}") for q in range(NQ)]
            ygate = [ap.tile([128, L], BF16, tag=f"yg{q}", name=f"yg{q}") for q in range(NQ)]
            b_rep = ap.tile([128, L], BF16, tag="brep")
            c_rep = ap.tile([128, L], BF16, tag="crep")
            carry = [ap.tile([128, 8], BF16, tag=f"carry{i}", name=f"carry{i}")
                     for i in range(2 * NQ)]
            cc_in = [dp.tile([96, LH], F32, name=f"cc_in{h}") for h in range(2)]
            cc_out = [dp.tile([96, LH], F32, name=f"cc_out{h}") for h in range(2)]

            for q in range(NQ):
                nc.vector.memset(xin[q][:, 0:3], 0.0)

            def hs(h):
                return slice(h * LH, (h + 1) * LH)

            def xs(h):
                return slice(3 + h * LH, 3 + (h + 1) * LH)

            # ================= phases 1-3 for one half =================
            def phases123(h):
                # in_proj xin chunks + conv, interleaved per q (AR critical path)
                for q in range(NQ):
                    ps = pss.tile([128, LH], F32, tag="ps")
                    for k in range(8):
                        nc.tensor.matmul(
                            ps[:],
                            w_in_sb[:, q * 128 + k * 1024:(q + 1) * 128 + k * 1024],
                            xT_sb[:, k * L + h * LH:k * L + h * LH + LH],
                            start=(k == 0), stop=(k == 7))
                    nc.scalar.copy(xin[q][:, xs(h)], ps[:])
                    psc = pss.tile([128, LH], F32, tag="ps")
                    for k in range(4):
                        nc.tensor.matmul(
                            psc[:],
                            convd_sb[:, (q * 4 + k) * 128:(q * 4 + k + 1) * 128],
                            xin[q][:, k + h * LH:k + h * LH + LH],
                            start=(k == 0), stop=(k == 3))
                    nc.scalar.activation(u[q][:, hs(h)], psc[:], AF.Silu,
                                         bias=convb_sb[:, q:q + 1])
                # x_proj partial -> DRAM -> AllReduce
                psx = pss.tile([96, LH], F32, tag="ps")
                for q in range(NQ):
                    nc.tensor.matmul(psx[:], w_xp_sb[:, q * 96:(q + 1) * 96],
                                     u[q][:, hs(h)], start=(q == 0), stop=(q == 3))
                xpc = sa.tile([96, LH], F32, tag="xpc")
                nc.scalar.copy(xpc[:], psx[:])
                nc.sync.dma_start(cc_in[h][:], xpc[:])
                if sim:
                    nc.sync.dma_start(cc_out[h][:], cc_in[h][:])
                else:
                    nc.gpsimd.collective_compute(
                        "AllReduce", OP.add,
                        replica_groups=[[0, 1, 2, 3], [4, 5, 6, 7]],
                        ins=[cc_in[h].opt()], outs=[cc_out[h].opt()])
                # z half (not needed until ygate; runs during AR)
                for m in range(4, 8):
                    ps = pss.tile([128, LH], F32, tag="ps")
                    for k in range(8):
                        nc.tensor.matmul(
                            ps[:],
                            w_in_sb[:, m * 128 + k * 1024:(m + 1) * 128 + k * 1024],
                            xT_sb[:, k * L + h * LH:k * L + h * LH + LH],
                            start=(k == 0), stop=(k == 7))
                    nc.scalar.activation(silu_z[m - 4][:, hs(h)], ps[:], AF.Silu)

            # ================= post-AR: phases 4-6 for one half =================
            def phase4(h):
                xdbl = sa.tile([96, LH], F32, tag="xdbl")
                nc.sync.dma_start(xdbl[:], cc_out[h][:])
                dt_bf = sa.tile([64, LH], BF16, tag="dtbf")
                nc.vector.tensor_copy(dt_bf[:], xdbl[0:64, :])
                bc_bf = sa.tile([32, LH], BF16, tag="bcbf")
                nc.vector.tensor_copy(bc_bf[:], xdbl[64:96, :])
                for which, dest in ((0, b_rep), (1, c_rep)):
                    ps = pss.tile([128, LH], F32, tag="ps")
                    nc.tensor.matmul(ps[:], selbc_sb[:, which * 128:(which + 1) * 128],
                                     bc_bf[:], start=True, stop=True)
                    nc.scalar.copy(dest[:, hs(h)], ps[:])
                for q in range(NQ):
                    ps = pss.tile([128, LH], F32, tag="ps")
                    nc.tensor.matmul(ps[:], w_dt_sb[:, q * 128:(q + 1) * 128],
                                     dt_bf[:], start=True, stop=True)
                    spe = gp.tile([128, LH], F32, tag="spe")
                    nc.scalar.activation(spe[:], ps[:], AF.Exp, bias=dtb_sb[:, q:q + 1])
                    nc.scalar.activation(delta[q][:, hs(h)], spe[:], AF.Ln, bias=1.0)
                    nc.vector.tensor_tensor(du[q][:, hs(h)], delta[q][:, hs(h)],
                                            u[q][:, hs(h)], op=OP.mult)

            def phase5_qhf(h, q, hf, yps):
                # replicate delta (-> exp -> a) and du via the same selector
                # stationary; du evicts through Act copy (same act table as Exp)
                arep = gp.tile([128, 8 * LH], BF16, tag="arep", name=f"a{q}_{hf}_{h}")
                durep = gp.tile([128, 8 * LH], BF16, tag="durep", name=f"d{q}_{hf}_{h}")
                for jj in range(8):
                    j = hf * 8 + jj
                    g = q * 16 + j
                    psd = psr.tile([128, LH], F32, tag="ps_rep")
                    nc.tensor.matmul(psd[:], selr_sb[:, j * 128:(j + 1) * 128],
                                     delta[q][:, hs(h)], start=True, stop=True)
                    psd2 = psr.tile([128, LH], F32, tag="ps_rep")
                    nc.tensor.matmul(psd2[:], selr_sb[:, j * 128:(j + 1) * 128],
                                     du[q][:, hs(h)], start=True, stop=True)
                    nc.scalar.activation(arep[:, jj * LH:(jj + 1) * LH], psd[:],
                                         AF.Exp, bias=0.0, scale=acol_sb[:, g:g + 1])
                    nc.scalar.copy(durep[:, jj * LH:(jj + 1) * LH], psd2[:])
                # bu = durep * B (broadcast over group-blocks), in place
                nc.vector.tensor_tensor(
                    durep[:].rearrange("p (j l) -> p j l", j=8),
                    durep[:].rearrange("p (j l) -> p j l", j=8),
                    b_rep[:, hs(h)].unsqueeze(1).broadcast_to([128, 8, LH]),
                    op=OP.mult)
                a3 = arep[:].rearrange("p (j l) -> p j l", l=LH)
                bu3 = durep[:].rearrange("p (j l) -> p j l", l=LH)
                cr = carry[q * 2 + hf]
                if h == 0:
                    # fresh segments: a=0 at starts of j>=1 (j=0 uses initial=0)
                    nc.vector.memset(a3[:, 1:8, 0:1], 0.0)
                else:
                    # fold carry into first bu of each segment, then reset a
                    tmp = sa.tile([128, 8], BF16, tag="tmp")
                    nc.vector.tensor_tensor(tmp[:].unsqueeze(2), a3[:, :, 0:1],
                                            cr[:].unsqueeze(2), op=OP.mult)
                    nc.vector.tensor_tensor(bu3[:, :, 0:1], bu3[:, :, 0:1],
                                            tmp[:].unsqueeze(2), op=OP.add)
                    nc.vector.memset(a3[:, :, 0:1], 0.0)
                hrep = gp.tile([128, 8 * LH], BF16, tag="hrep", name=f"h{q}_{hf}_{h}")
                nc.vector.tensor_tensor_scan(hrep[:], arep[:], durep[:],
                                             0.0, OP.mult, OP.add)
                h3 = hrep[:].rearrange("p (j l) -> p j l", l=LH)
                if h == 0:
                    nc.vector.tensor_copy(cr[:].unsqueeze(2), h3[:, :, LH - 1:LH])
                # g = h * C in place
                nc.vector.tensor_tensor(
                    h3, h3,
                    c_rep[:, hs(h)].unsqueeze(1).broadcast_to([128, 8, LH]),
                    op=OP.mult)
                for jj in range(8):
                    j = hf * 8 + jj
                    nc.tensor.matmul(yps[:], selo_sb[:, j * 128:(j + 1) * 128],
                                     hrep[:, jj * LH:(jj + 1) * LH],
                                     start=(hf == 0 and jj == 0),
                                     stop=(hf == 1 and jj == 7),
                                     skip_group_check=True)

            def ygate_q(h, q, yps):
                t1 = gp.tile([128, LH], F32, tag="t1")
                nc.vector.scalar_tensor_tensor(
                    t1[:], u[q][:, hs(h)], dcol_sb[:, q:q + 1], yps[:],
                    op0=OP.mult, op1=OP.add)
                nc.vector.tensor_tensor(ygate[q][:, hs(h)], t1[:],
                                        silu_z[q][:, hs(h)], op=OP.mult)

            def outproj(h):
                for m in range(8):
                    ps = pss.tile([128, LH], F32, tag="ps")
                    for q in range(NQ):
                        nc.tensor.matmul(
                            ps[:],
                            w_out_sb[:, q * 1024 + m * 128:q * 1024 + (m + 1) * 128],
                            ygate[q][:, hs(h)], start=(q == 0), stop=(q == 3))
                    ot = gp.tile([128, LH], BF16, tag="ot")
                    nc.scalar.copy(ot[:], ps[:])
                    nc.sync.dma_start(out.ap()[m * 128:(m + 1) * 128, hs(h)], ot[:])

            # ================= schedule =================
            phases123(0)
            phases123(1)          # runs on PE while AR(0) is in flight
            for h in range(2):
                phase4(h)
                pending_outproj = None
                for q in range(NQ):
                    yps = psy.tile([128, LH], F32, tag="ps_y", name=f"yps{q}_{h}")
                    for hf in range(2):
                        phase5_qhf(h, q, hf, yps)
                        # fill the PE queue behind the first selr batch of this
                        # half with the previous half's out_proj
                        if pending_outproj is None and h == 1 and q == 0 and hf == 0:
                            pending_outproj = True
                            outproj(0)
                    ygate_q(h, q, yps)
            outproj(1)

    nc.compile()
    return nc


def _prep_core_inputs(c, x, in_proj_w, conv_w, conv_b, x_proj_w, dt_proj_w,
                      dt_proj_b, A_log, D, out_proj_w, sel_r, sel_o, sel_bc):
    b, s = divmod(c, 4)
    sl = slice(s * DL, (s + 1) * DL)
    bf = ml_dtypes.bfloat16
    A = (-np.exp(A_log[sl])).astype(np.float32)            # [512, 16]
    # a_cols[:, g][p] = A[g*8 + p%8, p//8]  (n-major partitions)
    a_cols = np.empty((128, NGRP), np.float32)
    p = np.arange(128)
    for g in range(NGRP):
        a_cols[:, g] = A[g * 8 + (p % 8), p // 8]
    w_in_loc = np.concatenate([in_proj_w[sl], in_proj_w[2048 + s * DL:2048 + (s + 1) * DL]], 0)
    # conv as 16 diagonal [128,128] matrices (chunk-major, tap-minor)
    convd = np.zeros((128, NQ * 4 * 128), np.float32)
    cw = conv_w[sl, 0, :]                                  # [512, 4]
    for q in range(NQ):
        for k in range(4):
            blk = (q * 4 + k) * 128
            convd[np.arange(128), blk + np.arange(128)] = cw[q * 128:(q + 1) * 128, k]
    return {
        "xT": np.ascontiguousarray(x[b].T).astype(bf),
        "w_in": np.ascontiguousarray(w_in_loc.T).astype(bf),
        "w_xp": np.ascontiguousarray(x_proj_w[:, sl].T).astype(bf),
        "w_dt": np.ascontiguousarray(dt_proj_w[sl].T).astype(bf),
        "dt_b": np.ascontiguousarray(dt_proj_b[sl].reshape(NQ, 128).T).astype(np.float32),
        "w_out": np.ascontiguousarray(out_proj_w[:, sl].T).astype(bf),
        "a_cols": a_cols,
        "d_col": np.ascontiguousarray(D[sl].reshape(NQ, 128).T).astype(np.float32),
        "convd": convd.astype(bf),
        "convb": np.ascontiguousarray(conv_b[sl].reshape(NQ, 128).T).astype(np.float32),
        "selr": sel_r,
        "selo": sel_o,
        "selbc": sel_bc,
    }


def _selectors():
    bf = ml_dtypes.bfloat16
    p = np.arange(128)
    sel_r = np.zeros((128, 16 * 128), dtype=bf)
    sel_o = np.zeros((128, 16 * 128), dtype=bf)
    for j in range(16):
        sel_r[j * 8 + (p % 8), j * 128 + p] = 1       # replicate 8 ch -> (n,d)
        sel_o[p, j * 128 + j * 8 + (p % 8)] = 1       # reduce states back
    sel_bc = np.zeros((32, 2 * 128), dtype=bf)
    sel_bc[p // 8, p] = 1                              # B: rows 0:16 -> n-major
    sel_bc[16 + p // 8, 128 + p] = 1                   # C: rows 16:32
    return sel_r, sel_o, sel_bc


def kernel(x, in_proj_w, conv_w, conv_b, x_proj_w, dt_proj_w, dt_proj_b,
           A_log, D, out_proj_w):
    sel_r, sel_o, sel_bc = _selectors()
    if "nc" not in _CACHE:
        _CACHE["nc"] = _build()
    nc = _CACHE["nc"]

    args = (x, in_proj_w, conv_w, conv_b, x_proj_w, dt_proj_w, dt_proj_b,
            A_log, D, out_proj_w)
    in_maps = [_prep_core_inputs(c, *args, sel_r, sel_o, sel_bc) for c in range(8)]
    res = bass_utils.run_bass_kernel_spmd(nc, in_maps, core_ids=list(range(8)))
    outs = res.results
    _CACHE["last_result"] = res

    full = np.zeros((2, L, DM), dtype=np.float32)
    for b in range(2):
        acc = outs[4 * b]["out"].astype(np.float32)
        for s in range(1, 4):
            acc = acc + outs[4 * b + s]["out"].astype(np.float32)
        full[b] = acc.T
    return full
